# revision 22
# baseline (speedup 1.0000x reference)
"""Trainium2 Bass kernel: CrossAttentionBlock, data-parallel over batch on 8 NeuronCores.

Per-core computation (one batch element b):
    Q = query[b] @ Wq.T + bq          [1024, 512]
    K = key[b]   @ Wk.T + bk          [2048, 512]
    V = key[b]   @ Wv.T + bv          [2048, 512]
    S = Q @ K.T / sqrt(512)           [1024, 2048]
    out = softmax(S, axis=-1) @ V     [1024, 512]

Key algebraic fusion (saves the whole K projection, 64 matmuls/core):
    S_ij = q_i (Wq^T Wk) k_j + u_i + w_j + const,  u_i = q_i Wq^T bk
    Softmax over j is invariant to u_i and const, so with host-precomputed
    M = Wq^T Wk and w = (key @ Wk^T bq) * scale:
        softmax(S/sqrt(d)) == softmax((q M k^T)*scale + w)
    w folds into the exp as ScalarE's free per-partition bias.

Device-side layout ("transposed world": the TensorEngine contraction dim is
always on SBUF partitions, no on-device transposes needed). All matmul
operands fp16 (same PE rate as fp32r, half the DMA/SBUF traffic; end-to-end
rel err ~8e-4 vs 2e-2 budget):
  - T^T[e,i]  = sum_d M[d,e] qT[d,i]       (PE, accumulate over 4 d-subtiles)
  - V[k,e]    = sum_d kT[d,k] WvT[d,e]
  - S^T[k,i]  = sum_d kT[d,k] T^T[d,i]     (per 128-row k-tile, 512-col i-chunk)
  - E = exp(S^T * scale + w[k])            (ScalarE; no max-subtraction needed:
                                            E max ~4e3 fits fp16 range)
  - sumE[p,i] += E[p,i] for k-tiles 0..14  (VectorE accumulate, fp32)
  - sum_ps = ones.T @ sumE + ones.T @ E15  (two PE passes issued before the
                                            last att matmuls so the reciprocal
                                            overlaps them)
  - att^T[e,i] = sum_k V[k,e] E[k,i]       (PE, accumulated over k-tiles)
  - out^T = att^T * recip(sumexp)          (DVE reciprocal_approx_fast + muls)
Host transposes out^T back to [1024, 512] per batch element.
"""

import numpy as np

import concourse.bass as bass
import concourse.mybir as mybir
import concourse.tile as tile
from concourse import bacc
from concourse.bass_utils import run_bass_kernel_spmd

P = 128
D_MODEL = 512
DT = D_MODEL // P      # contraction subtiles (4)
ET = D_MODEL // P      # model-dim output tiles (4)
LQ = 1024
LK = 2048
NKT = LK // P          # key tiles (16)
F = 512                # matmul free dim / query-chunk width
NIC = LQ // F          # query chunks (2)
N_CORES = 8
SCALE = float(D_MODEL) ** -0.5

f32 = mybir.dt.float32
f32r = mybir.dt.float32r
f16 = mybir.dt.float16
AF = mybir.ActivationFunctionType

MMD = f16              # matmul operand dtype


def build_nc():
    # Bacc: its compile() pass splits multi-sem waits into EventSemaphores
    # (walrus allows only ONE sync wait per engine instruction).
    nc = bacc.Bacc()
    qT = nc.declare_dram_parameter("qT", [D_MODEL, LQ], MMD, isOutput=False)
    kT = nc.declare_dram_parameter("kT", [D_MODEL, LK], MMD, isOutput=False)
    m = nc.declare_dram_parameter("m", [D_MODEL, D_MODEL], MMD, isOutput=False)
    wvT = nc.declare_dram_parameter("wvT", [D_MODEL, D_MODEL], MMD, isOutput=False)
    wbias = nc.declare_dram_parameter("wbias", [P, NKT], f32, isOutput=False)
    bvB = nc.declare_dram_parameter("bvB", [P, D_MODEL], f16, isOutput=False)
    outT = nc.declare_dram_parameter("outT", [D_MODEL, LQ], f16, isOutput=True)

    qT_r = qT.rearrange("(dt p) i -> p dt i", p=P)
    kT_r = kT.rearrange("(dt p) k -> p dt k", p=P)
    m_r = m.rearrange("(dt p) e -> p dt e", p=P)
    wv_r = wvT.rearrange("(dt p) e -> p dt e", p=P)
    outT_r = outT.rearrange("(et p) i -> p et i", p=P)

    with (
        tile.TileContext(nc) as tc,
        tc.tile_pool(name="big", bufs=1) as big,
        tc.tile_pool(name="work", bufs=3) as work,
        tc.tile_pool(name="mmp", bufs=4, space="PSUM") as mmp,
        tc.tile_pool(name="attp", bufs=4, space="PSUM") as attp,
    ):
        qT_sb = big.tile([P, DT, LQ], MMD, tag="qT")
        kT_sb = big.tile([P, DT, LK], MMD, tag="kT")
        m_sb = big.tile([P, DT, D_MODEL], MMD, tag="m")
        wv_sb = big.tile([P, DT, D_MODEL], MMD, tag="wv")
        w_sb = big.tile([P, NKT], f32, tag="wbias")
        bv_sb = big.tile([P, D_MODEL], f16, tag="bv")
        T_sb = big.tile([P, DT, LQ], MMD, tag="T")
        V_sb = big.tile([P, NKT, D_MODEL], MMD, tag="V")
        out_sb = big.tile([P, ET, LQ], f16, tag="out")
        ones32_sb = big.tile([P, P], f32r, tag="ones32")
        ones16_sb = big.tile([P, P], f16, tag="ones16")

        # all-ones stationary tiles generated on-device (saves a DMA slot);
        # f32r memset goes through an f32 view (f32r has no memset value
        # type); on GpSimd since its preamble drains ~0.6us before Vector's
        nc.gpsimd.memset(ones32_sb[:].bitcast(f32), 1.0)
        nc.gpsimd.memset(ones16_sb[:], 1.0)
        warm_sb = big.tile([P, F], f16, tag="warm")
        nc.gpsimd.memset(warm_sb[:], 1.0)

        # ---- input DMAs, ordered so the first matmuls' deps land first;
        # single queue (a second parallel queue just steals bandwidth from
        # the critical m+qT prefix). m and qT split by d-subtile pairs so
        # T-proj can start its d0/d1 accumulation before d2/d3 arrive.
        # m rides the Scalar engine's DGE queue concurrently with qT on the
        # Sync queue, and qT itself is quartered so the very first matmuls
        # (T-proj ic0/d01, needing only m_d01 + qT[0:2, 0:512]) start after
        # ~0.5MB instead of ~1MB
        nc.scalar.dma_start(m_sb[:, 0:2, :], m_r[:, 0:2, :])
        nc.sync.dma_start(qT_sb[:, 0:2, 0:F], qT_r[:, 0:2, 0:F])
        nc.scalar.dma_start(m_sb[:, 2:4, :], m_r[:, 2:4, :])
        nc.sync.dma_start(qT_sb[:, 2:4, 0:F], qT_r[:, 2:4, 0:F])
        nc.sync.dma_start(qT_sb[:, 0:2, F:], qT_r[:, 0:2, F:])
        nc.sync.dma_start(qT_sb[:, 2:4, F:], qT_r[:, 2:4, F:])
        nc.scalar.dma_start(w_sb[:], wbias[:])
        nc.sync.dma_start(kT_sb[:, :, 0:LK // 2], kT_r[:, :, 0:LK // 2])
        nc.scalar.dma_start(wv_sb[:], wv_r)
        nc.scalar.dma_start(bv_sb[:], bvB[:])
        nc.sync.dma_start(kT_sb[:, :, LK // 2:], kT_r[:, :, LK // 2:])

        # ---- PE warm-up: throwaway matmuls during the DMA prologue keep the
        # HAM clock gate busy so it flips to 8/8 (2.4GHz) early; sized to end
        # just as the first real matmul's inputs land (~2us after issue)
        for wi in range(5):
            wps = mmp.tile([P, F], f32, tag="mm", name=f"warm{wi}")
            nc.tensor.matmul(wps[:], ones16_sb[:], warm_sb[:], start=True, stop=True)

        # ---- T = q @ M projection ----
        def t_proj_split(ic):
            # d0/d1 matmuls for all four et tiles first (only needs the first
            # m/qT half-DMAs), then d2/d3; holds 4 psum banks
            isl = slice(ic * F, (ic + 1) * F)
            pss = [
                mmp.tile([P, F], f32, tag="mm", name=f"ps_t{ic}{et}")
                for et in range(ET)
            ]
            for dh in range(2):
                for et in range(ET):
                    for d in (2 * dh, 2 * dh + 1):
                        nc.tensor.matmul(
                            pss[et][:],
                            m_sb[:, d, et * P:(et + 1) * P],
                            qT_sb[:, d, isl],
                            start=(d == 0),
                            stop=(d == DT - 1),
                        )
                    if dh == 1:
                        nc.scalar.activation(
                            T_sb[:, et, isl], pss[et][:], AF.Identity
                        )

        def t_proj(ic):
            isl = slice(ic * F, (ic + 1) * F)
            for et in range(ET):
                ps = mmp.tile([P, F], f32, tag="mm", name=f"ps_t{ic}{et}")
                for d in range(DT):
                    nc.tensor.matmul(
                        ps[:],
                        m_sb[:, d, et * P:(et + 1) * P],
                        qT_sb[:, d, isl],
                        start=(d == 0),
                        stop=(d == DT - 1),
                    )
                nc.scalar.activation(T_sb[:, et, isl], ps[:], AF.Identity)

        def v_proj(kc):
            for kt in range(4 * kc, 4 * kc + 4):
                ps = mmp.tile([P, F], f32, tag="mm", name=f"ps_v{kt}")
                for d in range(DT):
                    nc.tensor.matmul(
                        ps[:],
                        kT_sb[:, d, kt * P:(kt + 1) * P],
                        wv_sb[:, d, :],
                        start=(d == 0),
                        stop=(d == DT - 1),
                    )
                nc.vector.tensor_add(V_sb[:, kt, :], ps[:], bv_sb[:])

        t_proj_split(0)
        t_proj_split(1)
        v_proj(0)
        v_proj(1)
        v_proj(2)
        v_proj(3)

        # ---- attention ----
        for ic in range(NIC):
            isl = slice(ic * F, (ic + 1) * F)
            att = [
                attp.tile([P, F], f32, tag="att", name=f"att_{ic}_{j}")
                for j in range(ET)
            ]
            sumE = work.tile([P, F], f32r, tag="sumE", name=f"sumE_{ic}")

            def s_tile(kt, isl=isl):
                ps = mmp.tile([P, F], f32, tag="mm")
                for d in range(DT):
                    nc.tensor.matmul(
                        ps[:],
                        kT_sb[:, d, kt * P:(kt + 1) * P],
                        T_sb[:, d, isl],
                        start=(d == 0),
                        stop=(d == DT - 1),
                    )
                return ps

            # software-pipelined, depth 2: S(kt+1), S(kt+2) in flight while
            # exp(kt) runs on ScalarE (covers the ic-transition stall too)
            s_q = [s_tile(0), s_tile(1)]
            recip = work.tile([P, F], f32, tag="recip", name=f"recip_{ic}")
            for kt in range(NKT):
                if kt + 2 < NKT:
                    s_q.append(s_tile(kt + 2))
                E = work.tile([P, F], MMD, tag="E")
                nc.scalar.activation(
                    E[:], s_q.pop(0)[:], AF.Exp, scale=SCALE,
                    bias=w_sb[:, kt:kt + 1],
                )
                if kt == 0:
                    # row-sum accumulate on DVE (replaces 16 PE ones-matmuls)
                    nc.vector.tensor_copy(sumE[:], E[:])
                elif kt < NKT - 1:
                    nc.vector.tensor_add(sumE[:], sumE[:], E[:])
                else:
                    # total = ones.T@sumE(0..14) + ones.T@E15, issued BEFORE
                    # the final att matmuls so recip overlaps them on DVE
                    sum_ps = mmp.tile([P, F], f32, tag="mm", name=f"sum_{ic}")
                    nc.tensor.matmul(
                        sum_ps[:], ones32_sb[:], sumE[:], start=True, stop=False
                    )
                    nc.tensor.matmul(
                        sum_ps[:], ones16_sb[:], E[:], start=False, stop=True
                    )
                    nc.vector.reciprocal_approx_fast(recip[:], sum_ps[:])
                for et in range(ET):
                    nc.tensor.matmul(
                        att[et][:],
                        V_sb[:, kt, et * P:(et + 1) * P],
                        E[:],
                        start=(kt == 0),
                        stop=(kt == NKT - 1),
                    )
                    if kt == NKT - 1:
                        # normalize each att tile as soon as ITS final matmul
                        # stops (instead of after all four), recip is already
                        # done on DVE by now; two half-DMAs overlap the muls
                        nc.vector.tensor_mul(
                            out_sb[:, et, isl], att[et][:], recip[:]
                        )
                        if et % 2 == 1:
                            nc.sync.dma_start(
                                outT_r[:, et - 1:et + 1, isl],
                                out_sb[:, et - 1:et + 1, isl],
                            )

    nc.finalize()
    return nc


_NC_CACHE = None


def _get_nc():
    global _NC_CACHE
    if _NC_CACHE is None:
        _NC_CACHE = build_nc()
    return _NC_CACHE


def _prep_in_maps(query, key, Wq, bq, Wk, bk, Wv, bv):
    c = np.ascontiguousarray
    h = np.float16
    M = (Wq.T @ Wk).astype(h)          # S = q M k^T (+ bias terms, see header)
    a = Wk.T @ bq
    shared = {
        "m": c(M),
        "wvT": c(Wv.T.astype(h)),
        "bvB": c(np.broadcast_to(bv, (P, D_MODEL)).astype(h)),
    }
    maps = []
    for b in range(N_CORES):
        w = (key[b] @ a) * SCALE       # [LK] per-key softmax bias
        maps.append({
            "qT": c(query[b].T.astype(h)),
            "kT": c(key[b].T.astype(h)),
            "wbias": c(w.reshape(NKT, P).T.astype(np.float32)),
            **shared,
        })
    return maps


def kernel(**inputs):
    query = np.asarray(inputs["query"], np.float32)
    key = np.asarray(inputs["key"], np.float32)
    Wq = np.asarray(inputs["Wq"], np.float32)
    bq = np.asarray(inputs["bq"], np.float32)
    Wk = np.asarray(inputs["Wk"], np.float32)
    bk = np.asarray(inputs["bk"], np.float32)
    Wv = np.asarray(inputs["Wv"], np.float32)
    bv = np.asarray(inputs["bv"], np.float32)

    in_maps = _prep_in_maps(query, key, Wq, bq, Wk, bk, Wv, bv)
    res = run_bass_kernel_spmd(_get_nc(), in_maps, list(range(N_CORES)))
    global LAST_RES
    LAST_RES = res
    out = np.stack(
        [res.results[b]["outT"].astype(np.float32).T for b in range(N_CORES)]
    )
    return np.ascontiguousarray(out)


LAST_RES = None
